# revision 31
# baseline (speedup 1.0000x reference)
"""Causal multi-head attention (B=4, T=2048, C=1024, 16 heads) on 8 TRN2 NeuronCores.

Sharding: core (b, g) handles batch b and head-group g (8 heads = 512 features).
Each core projects Q/K/V for its own heads only (no duplicated projection work),
runs causal attention for its 8 heads over the full sequence, and computes a
PARTIAL output projection (contraction over its 512 ctx features). The host sums
the two partials per batch and adds the (bv-folded) output bias.

Causality: q-chunks of 512 attend to kv in [0, 512(j+1)); within the diagonal
512x512 block, score/exp/AV work is trimmed at 128 granularity and the
remaining triangular 128x128 blocks are masked with a single {0,1} tile.

Softmax denominators come FREE from the AV matmul: V carries a 65th all-ones
column, so PSUM row 64 of each ctx accumulator is l = sum_kv P[kv, q] (in
fp32).  No P-tile accumulation on DVE, no separate l matmuls.  The epilogue is
reciprocal (DVE) -> partition_broadcast to 64 rows (gpsimd) -> normalize (DVE).

The 1/sqrt(D) score scale is folded into the exp activation (scale=0.125), so
QT/KT hold unscaled q/k and fp8 weight tensors stay in the e4m3 normal range
with a plain x64 pre-scale.

Engine budget per core (bf16 matmuls, fp32 PSUM):
  PE  ~210us: QKVO projections + scores/AV head-pair packed (the roofline)
  ACT ~158us: exp only
  DVE ~100us: bias epilogues, masks, 1/l, normalize, PSUM->SBUF copies

Emission interleaves projection tiles as PE filler into the attention stream
(scores pipelined one step ahead of AV).
"""

import numpy as np
import ml_dtypes

B, T, C, NH, D = 4, 2048, 1024, 16, 64
P = 128
G = 8                 # heads per core
CH = 512              # q-chunk size
NCH = T // CH         # 4 q-chunks
KC = C // P           # 8 contraction chunks for QKV projections
OC = (C // 2) // P    # 4 contraction chunks for the partial O projection
NHP = G // 2          # 4 head pairs per core
DV = D + 1            # V columns incl. the ones column for l

_CACHE = {}


def _build():
    import concourse.bacc as bacc
    import concourse.tile as tile
    import concourse.mybir as mybir
    from concourse.bass import ts, ds

    f32 = mybir.dt.float32
    bf16 = mybir.dt.bfloat16
    f8 = mybir.dt.float8e4
    DR = mybir.MatmulPerfMode.DoubleRow
    EXP = mybir.ActivationFunctionType.Exp
    MUL = mybir.AluOpType.mult
    ADD = mybir.AluOpType.add

    nc = bacc.Bacc("TRN2", target_bir_lowering=False, debug=False, num_devices=8)

    def din(name, shape, dt=bf16):
        return nc.dram_tensor(name, list(shape), dt, kind="ExternalInput").ap()

    xT = din("xT", (C, T), f8)       # x^T for this batch (fp8: Q/K path only)
    xbT = din("xbT", (C, T))         # x^T in bf16 (V path: fp8 V noise does
                                     # not average out on short-context rows)
    wqT = din("wqT", (P, NHP, KC, P), f8)  # (Wq.T * 64), pre-permuted to the
                                     # SBUF tile layout [p, hp, k, m] so DMA
                                     # runs are contiguous per partition
    wkT = din("wkT", (P, NHP, KC, P), f8)  # Wk.T * 64, same layout
    wvT = din("wvT", (C, CH))        # Wv.T (bf16)
    woT = din("woT", (CH, C))        # Wo.T rows for this head group (bf16)
    bq = din("bq", (P, NHP), f32)    # bq, chunked per 128-feature block
    bk = din("bk", (P, NHP), f32)
    tri = din("tri", (P, P))         # {0,1}, tri[kv, q] = kv <= q
    out = nc.dram_tensor("out", [C, T], bf16, kind="ExternalOutput").ap()

    x_v = xT.rearrange("(k p) t -> p k t", p=P)      # [128, 8, 2048]
    xb_v = xbT.rearrange("(k p) t -> p k t", p=P)
    wv_v = wvT.rearrange("(k p) m -> p k m", p=P)
    wo_v = woT.rearrange("(k p) m -> p k m", p=P)    # [128, 4, 1024]

    from contextlib import ExitStack
    with ExitStack() as ctx:
        tc = ctx.enter_context(tile.TileContext(nc))

        consts = ctx.enter_context(tc.tile_pool(name="consts", bufs=1))
        big = ctx.enter_context(tc.tile_pool(name="big", bufs=1))
        ctxpool = ctx.enter_context(tc.tile_pool(name="ctxT", bufs=2))
        ptpool = ctx.enter_context(tc.tile_pool(name="pt", bufs=4))
        lrpool = ctx.enter_context(tc.tile_pool(name="lr", bufs=4))
        lbpool = ctx.enter_context(tc.tile_pool(name="lb", bufs=4))
        cspool = ctx.enter_context(tc.tile_pool(name="cs", bufs=6))
        opool = ctx.enter_context(tc.tile_pool(name="o", bufs=2))
        psumS = ctx.enter_context(tc.tile_pool(name="psumS", bufs=2, space="PSUM"))
        psumC = ctx.enter_context(tc.tile_pool(name="psumC", bufs=2, space="PSUM"))
        psumP = ctx.enter_context(tc.tile_pool(name="psumP", bufs=2, space="PSUM"))

        bq_sb = consts.tile([P, NHP], f32)
        bk_sb = consts.tile([P, NHP], f32)
        tri_sb = consts.tile([P, 1, P], bf16)
        ones64 = consts.tile([DV, 64], bf16)   # row 64 = ones (PE l-broadcast)
        nc.vector.memset(ones64[D : D + 1, :], 1.0)
        warm = consts.tile([1, 2], f32)
        nc.vector.memset(warm[:], 0.0)
        # preload the exp table set early
        nc.scalar.activation(warm[:], warm[:], EXP)

        X = big.tile([P, KC, T], f8)        # fp8 x (Q/K projections)
        XB = big.tile([P, KC, T], bf16)     # bf16 x (V projection)
        WQ = big.tile([P, NHP, KC, P], f8)
        WK = big.tile([P, NHP, KC, P], f8)
        WV = big.tile([P, KC, CH], bf16)
        WO = big.tile([P, OC, C], bf16)
        KT = big.tile([P, NHP, T], bf16)    # K^T  [d(2-head packed), hp, t]
        QT = big.tile([P, NHP, T], bf16)
        V = big.tile([P, T // P, G, DV], bf16)  # [kv_local, kv_chunk, head, d|1]
        nc.vector.memset(V[:, :, :, D : D + 1], 1.0)  # l column (LAST: ctx
                                                # stays on partitions 0..63,
                                                # l lands on partition 64)

        # DMA order = first-use order; X slice 0 + WK first (gate the first
        # matmul), weights on the sync queue, bulk X on gpsimd in parallel.
        # ALL bulk input loads go on the sync queue, in first-use order.  The
        # gpsimd queue is reserved for small latency-sensitive transfers
        # (epilogue l/cs shifts, output tiles) so they never sit behind
        # megabytes of input stream.
        nc.sync.dma_start(tri_sb[:, 0, :], tri)
        nc.sync.dma_start(bk_sb[:], bk)
        nc.sync.dma_start(bq_sb[:], bq)
        nc.sync.dma_start(X[:, :, 0:CH], x_v[:, :, 0:CH])
        nc.sync.dma_start(WK[:, 0], wkT[:, 0])
        for hp in range(1, NHP):
            nc.sync.dma_start(WK[:, hp], wkT[:, hp])
        nc.sync.dma_start(WQ[:], wqT)
        nc.gpsimd.dma_start(XB[:, :, 0:CH], xb_v[:, :, 0:CH])
        nc.sync.dma_start(WV[:], wv_v)
        nc.sync.dma_start(X[:, :, ds(CH, 3 * CH)], x_v[:, :, ds(CH, 3 * CH)])
        for tb in range(1, NCH):
            nc.sync.dma_start(XB[:, :, ds(CH * tb, CH)],
                              xb_v[:, :, ds(CH * tb, CH)])
        nc.sync.dma_start(WO[:], wo_v)

        # ---------- projection tile emitters (filler units) ----------
        # Q/K projections are fp8 DoubleRow over k-chunk pairs; weights were
        # pre-scaled by 64 on the host, descaled in the epilogue.
        def kt_tile(tb, hp):
            """KT[:, hp, 512*tb : ...] <- (Wk chunk)^T @ x chunk + bk."""
            ps = psumP.tile([P, CH], f32, tag="pp", name=f"pk{tb}{hp}")
            for k in range(0, KC, 2):
                nc.tensor.matmul(ps[:], WK[:, hp, k : k + 2, :],
                                 X[:, k : k + 2, ds(CH * tb, CH)],
                                 start=(k == 0), stop=(k == KC - 2),
                                 perf_mode=DR)
            nc.vector.tensor_scalar(
                out=KT[:, hp, ds(CH * tb, CH)], in0=ps[:],
                scalar1=1.0 / 64, scalar2=bk_sb[:, hp : hp + 1],
                op0=MUL, op1=ADD)

        def q_tile(j, hp):
            ps = psumP.tile([P, CH], f32, tag="pp", name=f"pq{j}{hp}")
            for k in range(0, KC, 2):
                nc.tensor.matmul(ps[:], WQ[:, hp, k : k + 2, :],
                                 X[:, k : k + 2, ds(CH * j, CH)],
                                 start=(k == 0), stop=(k == KC - 2),
                                 perf_mode=DR)
            nc.vector.tensor_scalar(
                out=QT[:, hp, ds(CH * j, CH)], in0=ps[:],
                scalar1=1.0 / 64, scalar2=bq_sb[:, hp : hp + 1],
                op0=MUL, op1=ADD)

        def v_tile(i):
            """V rows [128i : 128(i+1)] for all 8 heads (x chunk stationary)."""
            ps = psumP.tile([P, CH], f32, tag="pp", name=f"pv{i}")
            for k in range(KC):
                nc.tensor.matmul(ps[:], XB[:, k, ts(i, P)], WV[:, k, :],
                                 start=(k == 0), stop=(k == KC - 1))
            nc.vector.tensor_copy(V[:, i, :, 0:D],
                                  ps.rearrange("p (h d) -> p h d", d=D))

        COPY = mybir.ActivationFunctionType.Copy

        def o_tile(j, m, ctxT_j):
            """Partial out rows [128m:...], q chunk j (no bias; host adds it)."""
            ps = psumP.tile([P, CH], f32, tag="pp", name=f"po{j}{m}")
            for k in range(OC):
                nc.tensor.matmul(ps[:], WO[:, k, ts(m, P)], ctxT_j[:, k, :],
                                 start=(k == 0), stop=(k == OC - 1))
            o_sb = opool.tile([P, CH], bf16, tag="o", name=f"o{j}{m}")
            if j == NCH - 1:
                # tail: the exp stream is over, ACT is idle, and DVE is
                # draining epilogues -- copy on ACT to shorten the tail
                nc.scalar.activation(o_sb[:], ps[:], COPY)
            else:
                nc.vector.tensor_copy(o_sb[:], ps[:])
            nc.sync.dma_start(out[ts(m, P), ds(CH * j, CH)], o_sb[:])

        # ---------- attention ----------
        tri_b = tri_sb[:, 0:1, :].to_broadcast((P, 2, P))

        def attn_pair(hp, j, ctxT_j, fill):
            """Head pair hp, q rows [512j : 512(j+1)], kv in [0, 512(j+1))."""
            nkv = 4 * (j + 1)
            cps = [psumC.tile([DV, CH], f32, tag="ctx", name=f"cx{j}{hp}{hh}")
                   for hh in range(2)]
            pts = {}

            def score_step(c):
                m = c - 4 * j
                qo = P * m if m >= 0 else 0
                st = psumS.tile([P, 2, CH], f32, tag="st", name=f"st{j}{hp}{c}")
                for hh in range(2):
                    nc.tensor.matmul(
                        st[:, hh, qo:],
                        KT[ds(64 * hh, 64), hp, ts(c, P)],
                        QT[ds(64 * hh, 64), hp, ds(CH * j + qo, CH - qo)],
                        start=True, stop=True)
                pt = ptpool.tile([P, 2, CH], bf16, tag="pt", name=f"pt{j}{hp}{c}")
                nc.scalar.activation(pt[:, :, qo:], st[:, :, qo:], EXP,
                                     scale=0.125)
                if m >= 0:
                    nc.vector.tensor_tensor(pt[:, :, ds(qo, P)],
                                            pt[:, :, ds(qo, P)], tri_b, MUL)
                pts[c] = (pt, qo)

            def av_step(c):
                pt, qo = pts.pop(c)
                for hh in range(2):
                    nc.tensor.matmul(
                        cps[hh][:, qo:],
                        V[:, c, 2 * hp + hh, :],
                        pt[:, hh, qo:],
                        start=(c == 0), stop=(c == nkv - 1))

            # scores pipelined one step ahead of AV; fillers paced per step
            # (emitted between the score and AV pairs so filler streaming
            # covers the AV LDWEIGHTS)
            score_step(0)
            for c in range(1, nkv):
                score_step(c)
                fill()
                av_step(c - 1)
            fill()
            av_step(nkv - 1)

            # epilogue: l sits in PSUM partition 64 of each ctx accumulator.
            # HW constraints (micro-tested): the custom DVE reciprocal only
            # works at partition 0, DVE cannot move data across partitions,
            # and partition windows must be 32-aligned.
            # Two quick DVE copies evacuate ctx (fp32) and l (bf16) to SBUF
            # so the PSUM banks free right after the last AV.  A PE matmul
            # with a ones-row at partition 64 broadcasts l to partitions
            # 0..63 in PSUM (row-offset matmuls are the same mechanism the
            # hh=1 score matmuls use); reciprocal + normalize run aligned.
            for hh in range(2):
                csx = cspool.tile([DV, CH], f32, tag="cx", name=f"cq{j}{hp}{hh}")
                nc.vector.tensor_copy(csx[0:D, :], cps[hh][0:D, :])
                lsb = lrpool.tile([DV, CH], bf16, tag="l0", name=f"l0{j}{hp}{hh}")
                nc.vector.tensor_copy(lsb[D : D + 1, :], cps[hh][D : D + 1, :])
                lb_ps = psumP.tile([64, CH], f32, tag="pp", name=f"lp{j}{hp}{hh}")
                nc.tensor.matmul(lb_ps[:], ones64[D : D + 1, :],
                                 lsb[D : D + 1, :], start=True, stop=True)
                lbinv = lbpool.tile([64, CH], f32, tag="lb", name=f"lb{j}{hp}{hh}")
                nc.vector.reciprocal_approx_fast(lbinv[:], lb_ps[:])
                if hh == 0:
                    nc.vector.tensor_tensor(ctxT_j[0:64, hp, :],
                                            csx[0:D, :], lbinv[:], MUL)
                else:
                    cs1 = cspool.tile([64, CH], bf16, tag="cs",
                                      name=f"cs{j}{hp}")
                    nc.vector.tensor_tensor(cs1[:], csx[0:D, :], lbinv[:], MUL)
                    nc.gpsimd.dma_start(ctxT_j[ds(64, 64), hp, :], cs1[:])

        # ---------- schedule ----------
        # prologue: K/Q for block 0 first (their DMAs land first), then V
        for hp in range(NHP):
            kt_tile(0, hp)
        for hp in range(NHP):
            q_tile(0, hp)
        for i in range(NCH):
            v_tile(i)

        ctxT = [None] * NCH
        for j in range(NCH):
            ctxT[j] = ctxpool.tile([P, NHP, CH], bf16, tag="ctxT", name=f"cT{j}")
            # filler units due during attention chunk j
            units = []
            if j < NCH - 1:
                units += [lambda hp=hp: q_tile(j + 1, hp) for hp in range(NHP)]
                units += [lambda hp=hp: kt_tile(j + 1, hp) for hp in range(NHP)]
                units += [lambda i=i: v_tile(4 * (j + 1) + i) for i in range(4)]
            if j > 0:
                units += [lambda m=m, jj=j - 1: o_tile(jj, m, ctxT[jj])
                          for m in range(KC)]
            steps = NHP * 4 * (j + 1)
            state = {"s": 0, "f": 0}
            # hold back a few units past the last pair so PE has work during
            # the final epilogue chain of this chunk
            tail = 3 if units else 0
            paced = max(len(units) - tail, 0)

            def fill(units=units, steps=steps, state=state, paced=paced):
                state["s"] += 1
                want = (paced * state["s"] + steps - 1) // steps
                while state["f"] < want and state["f"] < paced:
                    units[state["f"]]()
                    state["f"] += 1

            for hp in range(NHP):
                attn_pair(hp, j, ctxT[j], fill)
            while state["f"] < len(units):
                units[state["f"]]()
                state["f"] += 1

        for m in range(KC):
            o_tile(NCH - 1, m, ctxT[NCH - 1])

    nc.compile()
    return nc


def _shard_inputs(x, Wq, bq, bk_, bv, bo, WqT, WkT, WvT, WoT):
    """Build the 8 per-core input maps. WqT is Wq.T/8; others are plain .T.

    Q/K path tensors go to the device in fp8e4m3; weights are pre-scaled by
    64 to clear the e4m3 denormal range, descaled on-device."""
    bf = ml_dtypes.bfloat16
    f8 = ml_dtypes.float8_e4m3
    tri = np.triu(np.ones((P, P), np.float32)).astype(bf)

    def perm_w(wT):  # (C, CH) -> [p, hp, k, m] (SBUF tile layout)
        return np.ascontiguousarray(
            wT.reshape(KC, P, NHP, P).transpose(1, 2, 0, 3))

    in_maps = []
    for b in range(B):
        xTb = np.ascontiguousarray(x[b].T)
        xT8 = xTb.astype(f8)
        xT16 = xTb.astype(bf)
        for g in range(2):
            sl = slice(CH * g, CH * (g + 1))
            in_maps.append({
                "xT": xT8,
                "xbT": xT16,
                "wqT": perm_w(WqT[:, sl] * 512.0).astype(f8),
                "wkT": perm_w(WkT[:, sl] * 64.0).astype(f8),
                "wvT": np.ascontiguousarray(WvT[:, sl]).astype(bf),
                "woT": np.ascontiguousarray(WoT[sl, :]).astype(bf),
                "bq": np.ascontiguousarray(bq[sl].reshape(NHP, P).T),
                "bk": np.ascontiguousarray(bk_[sl].reshape(NHP, P).T),
                "tri": tri,
            })
    return in_maps


def kernel(x, Wq, bq, Wk, bk, Wv, bv, Wo, bo):
    from concourse.bass_utils import run_bass_kernel_spmd

    x = np.asarray(x, np.float32)
    Wq = np.asarray(Wq, np.float32); bq = np.asarray(bq, np.float32)
    Wk = np.asarray(Wk, np.float32); bk = np.asarray(bk, np.float32)
    Wv = np.asarray(Wv, np.float32); bv = np.asarray(bv, np.float32)
    Wo = np.asarray(Wo, np.float32); bo = np.asarray(bo, np.float32)

    if "nc" not in _CACHE:
        _CACHE["nc"] = _build()
    nc = _CACHE["nc"]

    WqT = np.ascontiguousarray(Wq.T / 8.0)
    WkT = np.ascontiguousarray(Wk.T)
    WvT = np.ascontiguousarray(Wv.T)
    WoT = np.ascontiguousarray(Wo.T)
    in_maps = _shard_inputs(x, Wq, bq, bk, bv, bo, WqT, WkT, WvT, WoT)

    res = run_bass_kernel_spmd(nc, in_maps, core_ids=list(range(8)))
    bo_eff = (bo + Wo @ bv).astype(np.float32)
    outf = np.empty((B, T, C), np.float32)
    for b in range(B):
        o = (res.results[2 * b]["out"].astype(np.float32)
             + res.results[2 * b + 1]["out"].astype(np.float32))  # (C, T)
        outf[b] = o.T + bo_eff
    return outf


# revision 32
# speedup vs baseline: 1.0747x; 1.0747x over previous
"""Causal multi-head attention (B=4, T=2048, C=1024, 16 heads) on 8 TRN2 NeuronCores.

Sharding: core (b, g) handles batch b and head-group g (8 heads = 512 features).
Each core projects Q/K/V for its own heads only (no duplicated projection work),
runs causal attention for its 8 heads over the full sequence, and computes a
PARTIAL output projection (contraction over its 512 ctx features). The host sums
the two partials per batch and adds the (bv-folded) output bias.

Causality: q-chunks of 512 attend to kv in [0, 512(j+1)); within the diagonal
512x512 block, score/exp/AV work is trimmed at 128 granularity and the
remaining triangular 128x128 blocks are masked with a single {0,1} tile.

Softmax denominators come FREE from the AV matmul: V carries a 65th all-ones
column, so PSUM row 64 of each ctx accumulator is l = sum_kv P[kv, q] (in
fp32).  No P-tile accumulation on DVE, no separate l matmuls.  The epilogue is
reciprocal (DVE) -> partition_broadcast to 64 rows (gpsimd) -> normalize (DVE).

The 1/sqrt(D) score scale is folded into the exp activation (scale=0.125), so
QT/KT hold unscaled q/k and fp8 weight tensors stay in the e4m3 normal range
with a plain x64 pre-scale.

Engine budget per core (bf16 matmuls, fp32 PSUM):
  PE  ~210us: QKVO projections + scores/AV head-pair packed (the roofline)
  ACT ~158us: exp only
  DVE ~100us: bias epilogues, masks, 1/l, normalize, PSUM->SBUF copies

Emission interleaves projection tiles as PE filler into the attention stream
(scores pipelined one step ahead of AV).
"""

import numpy as np
import ml_dtypes

B, T, C, NH, D = 4, 2048, 1024, 16, 64
P = 128
G = 8                 # heads per core
CH = 512              # q-chunk size
NCH = T // CH         # 4 q-chunks
KC = C // P           # 8 contraction chunks for QKV projections
OC = (C // 2) // P    # 4 contraction chunks for the partial O projection
NHP = G // 2          # 4 head pairs per core
DV = D + 1            # V columns incl. the ones column for l

_CACHE = {}


def _build():
    import concourse.bacc as bacc
    import concourse.tile as tile
    import concourse.mybir as mybir
    from concourse.bass import ts, ds

    f32 = mybir.dt.float32
    bf16 = mybir.dt.bfloat16
    f8 = mybir.dt.float8e4
    DR = mybir.MatmulPerfMode.DoubleRow
    EXP = mybir.ActivationFunctionType.Exp
    MUL = mybir.AluOpType.mult
    ADD = mybir.AluOpType.add

    nc = bacc.Bacc("TRN2", target_bir_lowering=False, debug=False, num_devices=8)

    def din(name, shape, dt=bf16):
        return nc.dram_tensor(name, list(shape), dt, kind="ExternalInput").ap()

    xT = din("xT", (C, T), f8)       # x^T for this batch (fp8: Q/K path only)
    xbT = din("xbT", (C, T))         # x^T in bf16 (V path: fp8 V noise does
                                     # not average out on short-context rows)
    wqT = din("wqT", (P, NHP, KC, P), f8)  # (Wq.T * 64), pre-permuted to the
                                     # SBUF tile layout [p, hp, k, m] so DMA
                                     # runs are contiguous per partition
    wkT = din("wkT", (P, NHP, KC, P), f8)  # Wk.T * 64, same layout
    wvT = din("wvT", (C, CH))        # Wv.T (bf16)
    woT = din("woT", (CH, C))        # Wo.T rows for this head group (bf16)
    bq = din("bq", (P, NHP), f32)    # bq, chunked per 128-feature block
    bk = din("bk", (P, NHP), f32)
    tri = din("tri", (P, P))         # {0,1}, tri[kv, q] = kv <= q
    out = nc.dram_tensor("out", [C, T], bf16, kind="ExternalOutput").ap()

    x_v = xT.rearrange("(k p) t -> p k t", p=P)      # [128, 8, 2048]
    xb_v = xbT.rearrange("(k p) t -> p k t", p=P)
    wv_v = wvT.rearrange("(k p) m -> p k m", p=P)
    wo_v = woT.rearrange("(k p) m -> p k m", p=P)    # [128, 4, 1024]

    from contextlib import ExitStack
    with ExitStack() as ctx:
        tc = ctx.enter_context(tile.TileContext(nc))

        consts = ctx.enter_context(tc.tile_pool(name="consts", bufs=1))
        big = ctx.enter_context(tc.tile_pool(name="big", bufs=1))
        ctxpool = ctx.enter_context(tc.tile_pool(name="ctxT", bufs=2))
        ptpool = ctx.enter_context(tc.tile_pool(name="pt", bufs=4))
        lrpool = ctx.enter_context(tc.tile_pool(name="lr", bufs=4))
        lbpool = ctx.enter_context(tc.tile_pool(name="lb", bufs=4))
        cspool = ctx.enter_context(tc.tile_pool(name="cs", bufs=6))
        opool = ctx.enter_context(tc.tile_pool(name="o", bufs=2))
        psumS = ctx.enter_context(tc.tile_pool(name="psumS", bufs=2, space="PSUM"))
        psumC = ctx.enter_context(tc.tile_pool(name="psumC", bufs=2, space="PSUM"))
        psumP = ctx.enter_context(tc.tile_pool(name="psumP", bufs=2, space="PSUM"))

        bq_sb = consts.tile([P, NHP], f32)
        bk_sb = consts.tile([P, NHP], f32)
        tri_sb = consts.tile([P, 1, P], bf16)
        warm = consts.tile([1, 2], f32)
        nc.vector.memset(warm[:], 0.0)
        # preload the exp table set early
        nc.scalar.activation(warm[:], warm[:], EXP)

        X = big.tile([P, KC, T], f8)        # fp8 x (Q/K projections)
        XB = big.tile([P, KC, T], bf16)     # bf16 x (V projection)
        WQ = big.tile([P, NHP, KC, P], f8)
        WK = big.tile([P, NHP, KC, P], f8)
        WV = big.tile([P, KC, CH], bf16)
        WO = big.tile([P, OC, C], bf16)
        KT = big.tile([P, NHP, T], bf16)    # K^T  [d(2-head packed), hp, t]
        QT = big.tile([P, NHP, T], bf16)
        V = big.tile([P, T // P, G, DV], bf16)  # [kv_local, kv_chunk, head, d|1]
        nc.vector.memset(V[:, :, :, D : D + 1], 1.0)  # l column (LAST: ctx
                                                # stays on partitions 0..63,
                                                # l lands on partition 64)

        # DMA order = first-use order; X slice 0 + WK first (gate the first
        # matmul), weights on the sync queue, bulk X on gpsimd in parallel.
        # ALL bulk input loads go on the sync queue, in first-use order.  The
        # gpsimd queue is reserved for small latency-sensitive transfers
        # (epilogue l/cs shifts, output tiles) so they never sit behind
        # megabytes of input stream.
        nc.sync.dma_start(tri_sb[:, 0, :], tri)
        nc.sync.dma_start(bk_sb[:], bk)
        nc.sync.dma_start(bq_sb[:], bq)
        nc.sync.dma_start(X[:, :, 0:CH], x_v[:, :, 0:CH])
        nc.sync.dma_start(WK[:, 0], wkT[:, 0])
        for hp in range(1, NHP):
            nc.sync.dma_start(WK[:, hp], wkT[:, hp])
        nc.sync.dma_start(WQ[:], wqT)
        nc.gpsimd.dma_start(XB[:, :, 0:CH], xb_v[:, :, 0:CH])
        nc.sync.dma_start(WV[:], wv_v)
        nc.sync.dma_start(X[:, :, ds(CH, 3 * CH)], x_v[:, :, ds(CH, 3 * CH)])
        for tb in range(1, NCH):
            nc.sync.dma_start(XB[:, :, ds(CH * tb, CH)],
                              xb_v[:, :, ds(CH * tb, CH)])
        nc.sync.dma_start(WO[:], wo_v)

        # ---------- projection tile emitters (filler units) ----------
        # Q/K projections are fp8 DoubleRow over k-chunk pairs; weights were
        # pre-scaled by 64 on the host, descaled in the epilogue.
        def kt_tile(tb, hp):
            """KT[:, hp, 512*tb : ...] <- (Wk chunk)^T @ x chunk + bk."""
            ps = psumP.tile([P, CH], f32, tag="pp", name=f"pk{tb}{hp}")
            for k in range(0, KC, 2):
                nc.tensor.matmul(ps[:], WK[:, hp, k : k + 2, :],
                                 X[:, k : k + 2, ds(CH * tb, CH)],
                                 start=(k == 0), stop=(k == KC - 2),
                                 perf_mode=DR)
            nc.vector.tensor_scalar(
                out=KT[:, hp, ds(CH * tb, CH)], in0=ps[:],
                scalar1=1.0 / 64, scalar2=bk_sb[:, hp : hp + 1],
                op0=MUL, op1=ADD)

        def q_tile(j, hp):
            ps = psumP.tile([P, CH], f32, tag="pp", name=f"pq{j}{hp}")
            for k in range(0, KC, 2):
                nc.tensor.matmul(ps[:], WQ[:, hp, k : k + 2, :],
                                 X[:, k : k + 2, ds(CH * j, CH)],
                                 start=(k == 0), stop=(k == KC - 2),
                                 perf_mode=DR)
            nc.vector.tensor_scalar(
                out=QT[:, hp, ds(CH * j, CH)], in0=ps[:],
                scalar1=1.0 / 64, scalar2=bq_sb[:, hp : hp + 1],
                op0=MUL, op1=ADD)

        def v_tile(i):
            """V rows [128i : 128(i+1)] for all 8 heads (x chunk stationary)."""
            ps = psumP.tile([P, CH], f32, tag="pp", name=f"pv{i}")
            for k in range(KC):
                nc.tensor.matmul(ps[:], XB[:, k, ts(i, P)], WV[:, k, :],
                                 start=(k == 0), stop=(k == KC - 1))
            nc.vector.tensor_copy(V[:, i, :, 0:D],
                                  ps.rearrange("p (h d) -> p h d", d=D))

        COPY = mybir.ActivationFunctionType.Copy

        def o_tile(j, m, ctxT_j):
            """Partial out rows [128m:...], q chunk j (no bias; host adds it)."""
            ps = psumP.tile([P, CH], f32, tag="pp", name=f"po{j}{m}")
            for k in range(OC):
                nc.tensor.matmul(ps[:], WO[:, k, ts(m, P)], ctxT_j[:, k, :],
                                 start=(k == 0), stop=(k == OC - 1))
            o_sb = opool.tile([P, CH], bf16, tag="o", name=f"o{j}{m}")
            if j == NCH - 1:
                # tail: the exp stream is over, ACT is idle, and DVE is
                # draining epilogues -- copy on ACT to shorten the tail
                nc.scalar.activation(o_sb[:], ps[:], COPY)
            else:
                nc.vector.tensor_copy(o_sb[:], ps[:])
            nc.gpsimd.dma_start(out[ts(m, P), ds(CH * j, CH)], o_sb[:])

        # ---------- attention ----------
        tri_b = tri_sb[:, 0:1, :].to_broadcast((P, 2, P))

        def attn_pair(hp, j, ctxT_j, fill):
            """Head pair hp, q rows [512j : 512(j+1)], kv in [0, 512(j+1))."""
            nkv = 4 * (j + 1)
            cps = [psumC.tile([DV, CH], f32, tag="ctx", name=f"cx{j}{hp}{hh}")
                   for hh in range(2)]
            pts = {}

            def score_step(c):
                m = c - 4 * j
                qo = P * m if m >= 0 else 0
                st = psumS.tile([P, 2, CH], f32, tag="st", name=f"st{j}{hp}{c}")
                for hh in range(2):
                    nc.tensor.matmul(
                        st[:, hh, qo:],
                        KT[ds(64 * hh, 64), hp, ts(c, P)],
                        QT[ds(64 * hh, 64), hp, ds(CH * j + qo, CH - qo)],
                        start=True, stop=True)
                pt = ptpool.tile([P, 2, CH], bf16, tag="pt", name=f"pt{j}{hp}{c}")
                nc.scalar.activation(pt[:, :, qo:], st[:, :, qo:], EXP,
                                     scale=0.125)
                if m >= 0:
                    nc.vector.tensor_tensor(pt[:, :, ds(qo, P)],
                                            pt[:, :, ds(qo, P)], tri_b, MUL)
                pts[c] = (pt, qo)

            def av_step(c):
                pt, qo = pts.pop(c)
                for hh in range(2):
                    nc.tensor.matmul(
                        cps[hh][:, qo:],
                        V[:, c, 2 * hp + hh, :],
                        pt[:, hh, qo:],
                        start=(c == 0), stop=(c == nkv - 1))

            # scores pipelined one step ahead of AV; fillers paced per step
            # (emitted between the score and AV pairs so filler streaming
            # covers the AV LDWEIGHTS)
            score_step(0)
            for c in range(1, nkv):
                score_step(c)
                fill()
                av_step(c - 1)
            fill()
            av_step(nkv - 1)

            # epilogue: l sits in PSUM partition 64 of each ctx accumulator.
            # HW constraints (micro-tested): the custom DVE reciprocal and
            # gpsimd partition_broadcast only work with their source at
            # partition 0; only DMA moves data across partitions; partition
            # windows must be 32-aligned.
            # Two quick DVE copies evacuate ctx and l to SBUF so the PSUM
            # banks free up right after the last AV (the long l chain then
            # runs off the critical path): shift-DMA l to partition 0,
            # reciprocal, broadcast, normalize from the SBUF copy.
            for hh in range(2):
                csx = cspool.tile([DV, CH], f32, tag="cx", name=f"cq{j}{hp}{hh}")
                nc.vector.tensor_copy(csx[0:D, :], cps[hh][0:D, :])
                nc.vector.tensor_copy(csx[D : D + 1, :], cps[hh][D : D + 1, :])
                l0 = lrpool.tile([1, CH], f32, tag="l0", name=f"l0{j}{hp}{hh}")
                nc.gpsimd.dma_start(l0[:], csx[D : D + 1, :])
                linv = lrpool.tile([1, CH], f32, tag="li", name=f"li{j}{hp}{hh}")
                nc.vector.reciprocal_approx_fast(linv[:], l0[:])
                lb = lbpool.tile([64, CH], f32, tag="lb", name=f"lb{j}{hp}{hh}")
                nc.gpsimd.partition_broadcast(lb[:], linv[:], channels=64)
                if hh == 0:
                    nc.vector.tensor_tensor(ctxT_j[0:64, hp, :],
                                            csx[0:D, :], lb[:], MUL)
                else:
                    cs1 = cspool.tile([64, CH], bf16, tag="cs",
                                      name=f"cs{j}{hp}")
                    nc.vector.tensor_tensor(cs1[:], csx[0:D, :], lb[:], MUL)
                    nc.gpsimd.dma_start(ctxT_j[ds(64, 64), hp, :], cs1[:])

        # ---------- schedule ----------
        # prologue: K/Q for block 0 first (their DMAs land first), then V
        for hp in range(NHP):
            kt_tile(0, hp)
        for hp in range(NHP):
            q_tile(0, hp)
        for i in range(NCH):
            v_tile(i)

        ctxT = [None] * NCH
        for j in range(NCH):
            ctxT[j] = ctxpool.tile([P, NHP, CH], bf16, tag="ctxT", name=f"cT{j}")
            # filler units due during attention chunk j
            units = []
            if j < NCH - 1:
                units += [lambda hp=hp: q_tile(j + 1, hp) for hp in range(NHP)]
                units += [lambda hp=hp: kt_tile(j + 1, hp) for hp in range(NHP)]
                units += [lambda i=i: v_tile(4 * (j + 1) + i) for i in range(4)]
            if j > 0:
                units += [lambda m=m, jj=j - 1: o_tile(jj, m, ctxT[jj])
                          for m in range(KC)]
            steps = NHP * 4 * (j + 1)
            state = {"s": 0, "f": 0}
            # hold back a few units past the last pair so PE has work during
            # the final epilogue chain of this chunk
            tail = 3 if units else 0
            paced = max(len(units) - tail, 0)

            def fill(units=units, steps=steps, state=state, paced=paced):
                state["s"] += 1
                want = (paced * state["s"] + steps - 1) // steps
                while state["f"] < want and state["f"] < paced:
                    units[state["f"]]()
                    state["f"] += 1

            for hp in range(NHP):
                attn_pair(hp, j, ctxT[j], fill)
            while state["f"] < len(units):
                units[state["f"]]()
                state["f"] += 1

        for m in range(KC):
            o_tile(NCH - 1, m, ctxT[NCH - 1])

    nc.compile()
    return nc


def _shard_inputs(x, Wq, bq, bk_, bv, bo, WqT, WkT, WvT, WoT):
    """Build the 8 per-core input maps. WqT is Wq.T/8; others are plain .T.

    Q/K path tensors go to the device in fp8e4m3; weights are pre-scaled by
    64 to clear the e4m3 denormal range, descaled on-device."""
    bf = ml_dtypes.bfloat16
    f8 = ml_dtypes.float8_e4m3
    tri = np.triu(np.ones((P, P), np.float32)).astype(bf)

    def perm_w(wT):  # (C, CH) -> [p, hp, k, m] (SBUF tile layout)
        return np.ascontiguousarray(
            wT.reshape(KC, P, NHP, P).transpose(1, 2, 0, 3))

    in_maps = []
    for b in range(B):
        xTb = np.ascontiguousarray(x[b].T)
        xT8 = xTb.astype(f8)
        xT16 = xTb.astype(bf)
        for g in range(2):
            sl = slice(CH * g, CH * (g + 1))
            in_maps.append({
                "xT": xT8,
                "xbT": xT16,
                "wqT": perm_w(WqT[:, sl] * 512.0).astype(f8),
                "wkT": perm_w(WkT[:, sl] * 64.0).astype(f8),
                "wvT": np.ascontiguousarray(WvT[:, sl]).astype(bf),
                "woT": np.ascontiguousarray(WoT[sl, :]).astype(bf),
                "bq": np.ascontiguousarray(bq[sl].reshape(NHP, P).T),
                "bk": np.ascontiguousarray(bk_[sl].reshape(NHP, P).T),
                "tri": tri,
            })
    return in_maps


def kernel(x, Wq, bq, Wk, bk, Wv, bv, Wo, bo):
    from concourse.bass_utils import run_bass_kernel_spmd

    x = np.asarray(x, np.float32)
    Wq = np.asarray(Wq, np.float32); bq = np.asarray(bq, np.float32)
    Wk = np.asarray(Wk, np.float32); bk = np.asarray(bk, np.float32)
    Wv = np.asarray(Wv, np.float32); bv = np.asarray(bv, np.float32)
    Wo = np.asarray(Wo, np.float32); bo = np.asarray(bo, np.float32)

    if "nc" not in _CACHE:
        _CACHE["nc"] = _build()
    nc = _CACHE["nc"]

    WqT = np.ascontiguousarray(Wq.T / 8.0)
    WkT = np.ascontiguousarray(Wk.T)
    WvT = np.ascontiguousarray(Wv.T)
    WoT = np.ascontiguousarray(Wo.T)
    in_maps = _shard_inputs(x, Wq, bq, bk, bv, bo, WqT, WkT, WvT, WoT)

    res = run_bass_kernel_spmd(nc, in_maps, core_ids=list(range(8)))
    bo_eff = (bo + Wo @ bv).astype(np.float32)
    outf = np.empty((B, T, C), np.float32)
    for b in range(B):
        o = (res.results[2 * b]["out"].astype(np.float32)
             + res.results[2 * b + 1]["out"].astype(np.float32))  # (C, T)
        outf[b] = o.T + bo_eff
    return outf


# revision 33
# speedup vs baseline: 1.1052x; 1.0284x over previous
"""Causal multi-head attention (B=4, T=2048, C=1024, 16 heads) on 8 TRN2 NeuronCores.

Sharding: core (b, g) handles batch b and head-group g (8 heads = 512 features).
Each core projects Q/K/V for its own heads only (no duplicated projection work),
runs causal attention for its 8 heads over the full sequence, and computes a
PARTIAL output projection (contraction over its 512 ctx features). The host sums
the two partials per batch and adds the (bv-folded) output bias.

Causality: q-chunks of 512 attend to kv in [0, 512(j+1)); within the diagonal
512x512 block, score/exp/AV work is trimmed at 128 granularity and the
remaining triangular 128x128 blocks are masked with a single {0,1} tile.

Softmax denominators come FREE from the AV matmul: V carries a 65th all-ones
column, so PSUM row 64 of each ctx accumulator is l = sum_kv P[kv, q] (in
fp32).  No P-tile accumulation on DVE, no separate l matmuls.  The epilogue is
reciprocal (DVE) -> partition_broadcast to 64 rows (gpsimd) -> normalize (DVE).

The 1/sqrt(D) score scale is folded into the exp activation (scale=0.125), so
QT/KT hold unscaled q/k and fp8 weight tensors stay in the e4m3 normal range
with a plain x64 pre-scale.

Engine budget per core (bf16 matmuls, fp32 PSUM):
  PE  ~210us: QKVO projections + scores/AV head-pair packed (the roofline)
  ACT ~158us: exp only
  DVE ~100us: bias epilogues, masks, 1/l, normalize, PSUM->SBUF copies

Emission interleaves projection tiles as PE filler into the attention stream
(scores pipelined one step ahead of AV).
"""

import numpy as np
import ml_dtypes

B, T, C, NH, D = 4, 2048, 1024, 16, 64
P = 128
G = 8                 # heads per core
CH = 512              # q-chunk size
NCH = T // CH         # 4 q-chunks
KC = C // P           # 8 contraction chunks for QKV projections
OC = (C // 2) // P    # 4 contraction chunks for the partial O projection
NHP = G // 2          # 4 head pairs per core
DV = D + 1            # V columns incl. the ones column for l

_CACHE = {}


def _build():
    import concourse.bacc as bacc
    import concourse.tile as tile
    import concourse.mybir as mybir
    from concourse.bass import ts, ds

    f32 = mybir.dt.float32
    bf16 = mybir.dt.bfloat16
    f8 = mybir.dt.float8e4
    DR = mybir.MatmulPerfMode.DoubleRow
    EXP = mybir.ActivationFunctionType.Exp
    MUL = mybir.AluOpType.mult
    ADD = mybir.AluOpType.add

    nc = bacc.Bacc("TRN2", target_bir_lowering=False, debug=False, num_devices=8)

    def din(name, shape, dt=bf16):
        return nc.dram_tensor(name, list(shape), dt, kind="ExternalInput").ap()

    xT = din("xT", (C, T), f8)       # x^T for this batch (fp8: Q/K path only)
    xbT = din("xbT", (C, T))         # x^T in bf16 (V path: fp8 V noise does
                                     # not average out on short-context rows)
    wqT = din("wqT", (P, NHP, KC, P), f8)  # (Wq.T * 64), pre-permuted to the
                                     # SBUF tile layout [p, hp, k, m] so DMA
                                     # runs are contiguous per partition
    wkT = din("wkT", (P, NHP, KC, P), f8)  # Wk.T * 64, same layout
    wvT = din("wvT", (C, CH))        # Wv.T (bf16)
    woT = din("woT", (CH, C))        # Wo.T rows for this head group (bf16)
    bq = din("bq", (P, NHP), f32)    # bq, chunked per 128-feature block
    bk = din("bk", (P, NHP), f32)
    tri = din("tri", (P, P))         # {0,1}, tri[kv, q] = kv <= q
    out = nc.dram_tensor("out", [C, T], bf16, kind="ExternalOutput").ap()

    x_v = xT.rearrange("(k p) t -> p k t", p=P)      # [128, 8, 2048]
    xb_v = xbT.rearrange("(k p) t -> p k t", p=P)
    wv_v = wvT.rearrange("(k p) m -> p k m", p=P)
    wo_v = woT.rearrange("(k p) m -> p k m", p=P)    # [128, 4, 1024]

    from contextlib import ExitStack
    with ExitStack() as ctx:
        tc = ctx.enter_context(tile.TileContext(nc))

        consts = ctx.enter_context(tc.tile_pool(name="consts", bufs=1))
        big = ctx.enter_context(tc.tile_pool(name="big", bufs=1))
        ctxpool = ctx.enter_context(tc.tile_pool(name="ctxT", bufs=2))
        ptpool = ctx.enter_context(tc.tile_pool(name="pt", bufs=4))
        lrpool = ctx.enter_context(tc.tile_pool(name="lr", bufs=4))
        lbpool = ctx.enter_context(tc.tile_pool(name="lb", bufs=4))
        cspool = ctx.enter_context(tc.tile_pool(name="cs", bufs=6))
        opool = ctx.enter_context(tc.tile_pool(name="o", bufs=2))
        psumS = ctx.enter_context(tc.tile_pool(name="psumS", bufs=2, space="PSUM"))
        psumC = ctx.enter_context(tc.tile_pool(name="psumC", bufs=2, space="PSUM"))
        psumP = ctx.enter_context(tc.tile_pool(name="psumP", bufs=2, space="PSUM"))

        bq_sb = consts.tile([P, NHP], f32)
        bk_sb = consts.tile([P, NHP], f32)
        tri_sb = consts.tile([P, 1, P], bf16)
        warm = consts.tile([1, 2], f32)
        nc.vector.memset(warm[:], 0.0)
        # preload the exp table set early
        nc.scalar.activation(warm[:], warm[:], EXP)

        X = big.tile([P, KC, T], f8)        # fp8 x (Q/K projections)
        XB = big.tile([P, KC, T], bf16)     # bf16 x (V projection)
        WQ = big.tile([P, NHP, KC, P], f8)
        WK = big.tile([P, NHP, KC, P], f8)
        WV = big.tile([P, KC, CH], bf16)
        WO = big.tile([P, OC, C], bf16)
        KT = big.tile([P, NHP, T], bf16)    # K^T  [d(2-head packed), hp, t]
        QT = big.tile([P, NHP, T], bf16)
        V = big.tile([P, T // P, G, DV], bf16)  # [kv_local, kv_chunk, head, d|1]
        nc.vector.memset(V[:, :, :, D : D + 1], 1.0)  # l column (LAST: ctx
                                                # stays on partitions 0..63,
                                                # l lands on partition 64)

        # DMA order = first-use order; X slice 0 + WK first (gate the first
        # matmul), weights on the sync queue, bulk X on gpsimd in parallel.
        # ALL bulk input loads go on the sync queue, in first-use order.  The
        # gpsimd queue is reserved for small latency-sensitive transfers
        # (epilogue l/cs shifts, output tiles) so they never sit behind
        # megabytes of input stream.
        nc.sync.dma_start(tri_sb[:, 0, :], tri)
        nc.sync.dma_start(bk_sb[:], bk)
        nc.sync.dma_start(bq_sb[:], bq)
        nc.sync.dma_start(X[:, :, 0:CH], x_v[:, :, 0:CH])
        nc.sync.dma_start(WK[:, 0], wkT[:, 0])
        for hp in range(1, NHP):
            nc.sync.dma_start(WK[:, hp], wkT[:, hp])
        nc.sync.dma_start(WQ[:], wqT)
        nc.gpsimd.dma_start(XB[:, :, 0:CH], xb_v[:, :, 0:CH])
        nc.sync.dma_start(WV[:], wv_v)
        nc.sync.dma_start(X[:, :, ds(CH, 3 * CH)], x_v[:, :, ds(CH, 3 * CH)])
        for tb in range(1, NCH):
            nc.sync.dma_start(XB[:, :, ds(CH * tb, CH)],
                              xb_v[:, :, ds(CH * tb, CH)])
        nc.sync.dma_start(WO[:], wo_v)

        # ---------- projection tile emitters (filler units) ----------
        # Q/K projections are fp8 DoubleRow over k-chunk pairs; weights were
        # pre-scaled by 64 on the host, descaled in the epilogue.
        def kt_tile(tb, hp):
            """KT[:, hp, 512*tb : ...] <- (Wk chunk)^T @ x chunk + bk."""
            ps = psumP.tile([P, CH], f32, tag="pp", name=f"pk{tb}{hp}")
            for k in range(0, KC, 2):
                nc.tensor.matmul(ps[:], WK[:, hp, k : k + 2, :],
                                 X[:, k : k + 2, ds(CH * tb, CH)],
                                 start=(k == 0), stop=(k == KC - 2),
                                 perf_mode=DR)
            nc.vector.tensor_scalar(
                out=KT[:, hp, ds(CH * tb, CH)], in0=ps[:],
                scalar1=1.0 / 64, scalar2=bk_sb[:, hp : hp + 1],
                op0=MUL, op1=ADD)

        def q_tile(j, hp):
            ps = psumP.tile([P, CH], f32, tag="pp", name=f"pq{j}{hp}")
            for k in range(0, KC, 2):
                nc.tensor.matmul(ps[:], WQ[:, hp, k : k + 2, :],
                                 X[:, k : k + 2, ds(CH * j, CH)],
                                 start=(k == 0), stop=(k == KC - 2),
                                 perf_mode=DR)
            nc.vector.tensor_scalar(
                out=QT[:, hp, ds(CH * j, CH)], in0=ps[:],
                scalar1=1.0 / 64, scalar2=bq_sb[:, hp : hp + 1],
                op0=MUL, op1=ADD)

        def v_tile(i):
            """V rows [128i : 128(i+1)] for all 8 heads (x chunk stationary)."""
            ps = psumP.tile([P, CH], f32, tag="pp", name=f"pv{i}")
            for k in range(KC):
                nc.tensor.matmul(ps[:], XB[:, k, ts(i, P)], WV[:, k, :],
                                 start=(k == 0), stop=(k == KC - 1))
            nc.vector.tensor_copy(V[:, i, :, 0:D],
                                  ps.rearrange("p (h d) -> p h d", d=D))

        COPY = mybir.ActivationFunctionType.Copy

        def o_tile(j, m, ctxT_j):
            """Partial out rows [128m:...], q chunk j (no bias; host adds it)."""
            ps = psumP.tile([P, CH], f32, tag="pp", name=f"po{j}{m}")
            for k in range(OC):
                nc.tensor.matmul(ps[:], WO[:, k, ts(m, P)], ctxT_j[:, k, :],
                                 start=(k == 0), stop=(k == OC - 1))
            o_sb = opool.tile([P, CH], bf16, tag="o", name=f"o{j}{m}")
            if j == NCH - 1:
                # tail: the exp stream is over, ACT is idle, and DVE is
                # draining epilogues -- copy on ACT to shorten the tail
                nc.scalar.activation(o_sb[:], ps[:], COPY)
            else:
                nc.vector.tensor_copy(o_sb[:], ps[:])
            nc.sync.dma_start(out[ts(m, P), ds(CH * j, CH)], o_sb[:])

        # ---------- attention ----------
        tri_b = tri_sb[:, 0:1, :].to_broadcast((P, 2, P))

        def attn_pair(hp, j, ctxT_j, fill):
            """Head pair hp, q rows [512j : 512(j+1)], kv in [0, 512(j+1))."""
            nkv = 4 * (j + 1)
            cps = [psumC.tile([DV, CH], f32, tag="ctx", name=f"cx{j}{hp}{hh}")
                   for hh in range(2)]
            pts = {}

            def score_step(c):
                m = c - 4 * j
                qo = P * m if m >= 0 else 0
                st = psumS.tile([P, 2, CH], f32, tag="st", name=f"st{j}{hp}{c}")
                for hh in range(2):
                    nc.tensor.matmul(
                        st[:, hh, qo:],
                        KT[ds(64 * hh, 64), hp, ts(c, P)],
                        QT[ds(64 * hh, 64), hp, ds(CH * j + qo, CH - qo)],
                        start=True, stop=True)
                pt = ptpool.tile([P, 2, CH], bf16, tag="pt", name=f"pt{j}{hp}{c}")
                nc.scalar.activation(pt[:, :, qo:], st[:, :, qo:], EXP,
                                     scale=0.125)
                if m >= 0:
                    nc.vector.tensor_tensor(pt[:, :, ds(qo, P)],
                                            pt[:, :, ds(qo, P)], tri_b, MUL)
                pts[c] = (pt, qo)

            def av_step(c):
                pt, qo = pts.pop(c)
                for hh in range(2):
                    nc.tensor.matmul(
                        cps[hh][:, qo:],
                        V[:, c, 2 * hp + hh, :],
                        pt[:, hh, qo:],
                        start=(c == 0), stop=(c == nkv - 1))

            # scores pipelined one step ahead of AV; fillers paced per step
            # (emitted between the score and AV pairs so filler streaming
            # covers the AV LDWEIGHTS)
            score_step(0)
            for c in range(1, nkv):
                score_step(c)
                fill()
                av_step(c - 1)
            fill()
            av_step(nkv - 1)

            # epilogue: l sits in PSUM partition 64 of each ctx accumulator.
            # HW constraints (micro-tested): the custom DVE reciprocal and
            # gpsimd partition_broadcast only work with their source at
            # partition 0; only DMA moves data across partitions; partition
            # windows must be 32-aligned.
            # Two quick DVE copies evacuate ctx and l to SBUF so the PSUM
            # banks free up right after the last AV (the long l chain then
            # runs off the critical path): shift-DMA l to partition 0,
            # reciprocal, broadcast, normalize from the SBUF copy.
            # j=0 epilogues overlap the input DMA stream, so their small
            # transfers stay on the gpsimd queue; later ones use the (by
            # then idle) sync hardware queue for lower completion latency.
            qd = nc.gpsimd if j == 0 else nc.sync
            for hh in range(2):
                csx = cspool.tile([DV, CH], f32, tag="cx", name=f"cq{j}{hp}{hh}")
                # l row first: its shift-DMA is the longest pole of the chain
                nc.vector.tensor_copy(csx[D : D + 1, :], cps[hh][D : D + 1, :])
                l0 = lrpool.tile([1, CH], f32, tag="l0", name=f"l0{j}{hp}{hh}")
                qd.dma_start(l0[:], csx[D : D + 1, :])
                nc.vector.tensor_copy(csx[0:D, :], cps[hh][0:D, :])
                linv = lrpool.tile([1, CH], f32, tag="li", name=f"li{j}{hp}{hh}")
                nc.vector.reciprocal_approx_fast(linv[:], l0[:])
                lb = lbpool.tile([64, CH], f32, tag="lb", name=f"lb{j}{hp}{hh}")
                nc.gpsimd.partition_broadcast(lb[:], linv[:], channels=64)
                if hh == 0:
                    nc.vector.tensor_tensor(ctxT_j[0:64, hp, :],
                                            csx[0:D, :], lb[:], MUL)
                else:
                    cs1 = cspool.tile([64, CH], bf16, tag="cs",
                                      name=f"cs{j}{hp}")
                    nc.vector.tensor_tensor(cs1[:], csx[0:D, :], lb[:], MUL)
                    qd.dma_start(ctxT_j[ds(64, 64), hp, :], cs1[:])

        # ---------- schedule ----------
        # prologue: K/Q for block 0 first (their DMAs land first), then V
        for hp in range(NHP):
            kt_tile(0, hp)
        for hp in range(NHP):
            q_tile(0, hp)
        for i in range(NCH):
            v_tile(i)

        ctxT = [None] * NCH
        for j in range(NCH):
            ctxT[j] = ctxpool.tile([P, NHP, CH], bf16, tag="ctxT", name=f"cT{j}")
            # filler units due during attention chunk j
            units = []
            if j < NCH - 1:
                units += [lambda hp=hp: q_tile(j + 1, hp) for hp in range(NHP)]
                units += [lambda hp=hp: kt_tile(j + 1, hp) for hp in range(NHP)]
                units += [lambda i=i: v_tile(4 * (j + 1) + i) for i in range(4)]
            if j > 0:
                units += [lambda m=m, jj=j - 1: o_tile(jj, m, ctxT[jj])
                          for m in range(KC)]
            steps = NHP * 4 * (j + 1)
            state = {"s": 0, "f": 0}
            # hold back a few units past the last pair so PE has work during
            # the final epilogue chain of this chunk
            tail = (6 if j == NCH - 1 else 3) if units else 0
            paced = max(len(units) - tail, 0)

            def fill(units=units, steps=steps, state=state, paced=paced):
                state["s"] += 1
                want = (paced * state["s"] + steps - 1) // steps
                while state["f"] < want and state["f"] < paced:
                    units[state["f"]]()
                    state["f"] += 1

            for hp in range(NHP):
                attn_pair(hp, j, ctxT[j], fill)
            while state["f"] < len(units):
                units[state["f"]]()
                state["f"] += 1

        for m in range(KC):
            o_tile(NCH - 1, m, ctxT[NCH - 1])

    nc.compile()
    return nc


def _shard_inputs(x, Wq, bq, bk_, bv, bo, WqT, WkT, WvT, WoT):
    """Build the 8 per-core input maps. WqT is Wq.T/8; others are plain .T.

    Q/K path tensors go to the device in fp8e4m3; weights are pre-scaled by
    64 to clear the e4m3 denormal range, descaled on-device."""
    bf = ml_dtypes.bfloat16
    f8 = ml_dtypes.float8_e4m3
    tri = np.triu(np.ones((P, P), np.float32)).astype(bf)

    def perm_w(wT):  # (C, CH) -> [p, hp, k, m] (SBUF tile layout)
        return np.ascontiguousarray(
            wT.reshape(KC, P, NHP, P).transpose(1, 2, 0, 3))

    in_maps = []
    for b in range(B):
        xTb = np.ascontiguousarray(x[b].T)
        xT8 = xTb.astype(f8)
        xT16 = xTb.astype(bf)
        for g in range(2):
            sl = slice(CH * g, CH * (g + 1))
            in_maps.append({
                "xT": xT8,
                "xbT": xT16,
                "wqT": perm_w(WqT[:, sl] * 512.0).astype(f8),
                "wkT": perm_w(WkT[:, sl] * 64.0).astype(f8),
                "wvT": np.ascontiguousarray(WvT[:, sl]).astype(bf),
                "woT": np.ascontiguousarray(WoT[sl, :]).astype(bf),
                "bq": np.ascontiguousarray(bq[sl].reshape(NHP, P).T),
                "bk": np.ascontiguousarray(bk_[sl].reshape(NHP, P).T),
                "tri": tri,
            })
    return in_maps


def kernel(x, Wq, bq, Wk, bk, Wv, bv, Wo, bo):
    from concourse.bass_utils import run_bass_kernel_spmd

    x = np.asarray(x, np.float32)
    Wq = np.asarray(Wq, np.float32); bq = np.asarray(bq, np.float32)
    Wk = np.asarray(Wk, np.float32); bk = np.asarray(bk, np.float32)
    Wv = np.asarray(Wv, np.float32); bv = np.asarray(bv, np.float32)
    Wo = np.asarray(Wo, np.float32); bo = np.asarray(bo, np.float32)

    if "nc" not in _CACHE:
        _CACHE["nc"] = _build()
    nc = _CACHE["nc"]

    WqT = np.ascontiguousarray(Wq.T / 8.0)
    WkT = np.ascontiguousarray(Wk.T)
    WvT = np.ascontiguousarray(Wv.T)
    WoT = np.ascontiguousarray(Wo.T)
    in_maps = _shard_inputs(x, Wq, bq, bk, bv, bo, WqT, WkT, WvT, WoT)

    res = run_bass_kernel_spmd(nc, in_maps, core_ids=list(range(8)))
    bo_eff = (bo + Wo @ bv).astype(np.float32)
    outf = np.empty((B, T, C), np.float32)
    for b in range(B):
        o = (res.results[2 * b]["out"].astype(np.float32)
             + res.results[2 * b + 1]["out"].astype(np.float32))  # (C, T)
        outf[b] = o.T + bo_eff
    return outf


# revision 34
# speedup vs baseline: 1.1092x; 1.0036x over previous
"""Causal multi-head attention (B=4, T=2048, C=1024, 16 heads) on 8 TRN2 NeuronCores.

Sharding: core (b, g) handles batch b and head-group g (8 heads = 512 features).
Each core projects Q/K/V for its own heads only (no duplicated projection work),
runs causal attention for its 8 heads over the full sequence, and computes a
PARTIAL output projection (contraction over its 512 ctx features). The host sums
the two partials per batch and adds the (bv-folded) output bias.

Causality: q-chunks of 512 attend to kv in [0, 512(j+1)); within the diagonal
512x512 block, score/exp/AV work is trimmed at 128 granularity and the
remaining triangular 128x128 blocks are masked with a single {0,1} tile.

Softmax denominators come FREE from the AV matmul: V carries a 65th all-ones
column, so PSUM row 64 of each ctx accumulator is l = sum_kv P[kv, q] (in
fp32).  No P-tile accumulation on DVE, no separate l matmuls.  The epilogue is
reciprocal (DVE) -> partition_broadcast to 64 rows (gpsimd) -> normalize (DVE).

The 1/sqrt(D) score scale is folded into the exp activation (scale=0.125), so
QT/KT hold unscaled q/k and fp8 weight tensors stay in the e4m3 normal range
with a plain x64 pre-scale.

Engine budget per core (bf16 matmuls, fp32 PSUM):
  PE  ~210us: QKVO projections + scores/AV head-pair packed (the roofline)
  ACT ~158us: exp only
  DVE ~100us: bias epilogues, masks, 1/l, normalize, PSUM->SBUF copies

Emission interleaves projection tiles as PE filler into the attention stream
(scores pipelined one step ahead of AV).
"""

import numpy as np
import ml_dtypes

B, T, C, NH, D = 4, 2048, 1024, 16, 64
P = 128
G = 8                 # heads per core
CH = 512              # q-chunk size
NCH = T // CH         # 4 q-chunks
KC = C // P           # 8 contraction chunks for QKV projections
OC = (C // 2) // P    # 4 contraction chunks for the partial O projection
NHP = G // 2          # 4 head pairs per core
DV = D + 1            # V columns incl. the ones column for l

_CACHE = {}


def _build():
    import concourse.bacc as bacc
    import concourse.tile as tile
    import concourse.mybir as mybir
    from concourse.bass import ts, ds

    f32 = mybir.dt.float32
    bf16 = mybir.dt.bfloat16
    f8 = mybir.dt.float8e4
    DR = mybir.MatmulPerfMode.DoubleRow
    EXP = mybir.ActivationFunctionType.Exp
    MUL = mybir.AluOpType.mult
    ADD = mybir.AluOpType.add

    nc = bacc.Bacc("TRN2", target_bir_lowering=False, debug=False, num_devices=8)

    def din(name, shape, dt=bf16):
        return nc.dram_tensor(name, list(shape), dt, kind="ExternalInput").ap()

    xT = din("xT", (C, T), f8)       # x^T for this batch (fp8: Q/K path only)
    xbT = din("xbT", (C, T))         # x^T in bf16 (V path: fp8 V noise does
                                     # not average out on short-context rows)
    wqT = din("wqT", (P, NHP, KC, P), f8)  # (Wq.T * 64), pre-permuted to the
                                     # SBUF tile layout [p, hp, k, m] so DMA
                                     # runs are contiguous per partition
    wkT = din("wkT", (P, NHP, KC, P), f8)  # Wk.T * 64, same layout
    wvT = din("wvT", (C, CH))        # Wv.T (bf16)
    woT = din("woT", (CH, C))        # Wo.T rows for this head group (bf16)
    bq = din("bq", (P, NHP), f32)    # bq, chunked per 128-feature block
    bk = din("bk", (P, NHP), f32)
    tri = din("tri", (P, P))         # {0,1}, tri[kv, q] = kv <= q
    out = nc.dram_tensor("out", [C, T], bf16, kind="ExternalOutput").ap()

    x_v = xT.rearrange("(k p) t -> p k t", p=P)      # [128, 8, 2048]
    xb_v = xbT.rearrange("(k p) t -> p k t", p=P)
    wv_v = wvT.rearrange("(k p) m -> p k m", p=P)
    wo_v = woT.rearrange("(k p) m -> p k m", p=P)    # [128, 4, 1024]

    from contextlib import ExitStack
    with ExitStack() as ctx:
        tc = ctx.enter_context(tile.TileContext(nc))

        consts = ctx.enter_context(tc.tile_pool(name="consts", bufs=1))
        big = ctx.enter_context(tc.tile_pool(name="big", bufs=1))
        ctxpool = ctx.enter_context(tc.tile_pool(name="ctxT", bufs=2))
        ptpool = ctx.enter_context(tc.tile_pool(name="pt", bufs=4))
        lrpool = ctx.enter_context(tc.tile_pool(name="lr", bufs=4))
        lbpool = ctx.enter_context(tc.tile_pool(name="lb", bufs=4))
        cspool = ctx.enter_context(tc.tile_pool(name="cs", bufs=6))
        opool = ctx.enter_context(tc.tile_pool(name="o", bufs=2))
        psumS = ctx.enter_context(tc.tile_pool(name="psumS", bufs=2, space="PSUM"))
        psumC = ctx.enter_context(tc.tile_pool(name="psumC", bufs=2, space="PSUM"))
        psumP = ctx.enter_context(tc.tile_pool(name="psumP", bufs=2, space="PSUM"))

        bq_sb = consts.tile([P, NHP], f32)
        bk_sb = consts.tile([P, NHP], f32)
        tri_sb = consts.tile([P, 1, P], bf16)
        warm = consts.tile([1, 2], f32)
        nc.vector.memset(warm[:], 0.0)
        # preload the exp table set early
        nc.scalar.activation(warm[:], warm[:], EXP)

        X = big.tile([P, KC, T], f8)        # fp8 x (Q/K projections)
        XB = big.tile([P, KC, T], bf16)     # bf16 x (V projection)
        WQ = big.tile([P, NHP, KC, P], f8)
        WK = big.tile([P, NHP, KC, P], f8)
        WV = big.tile([P, KC, CH], bf16)
        WO = big.tile([P, OC, C], bf16)
        KT = big.tile([P, NHP, T], bf16)    # K^T  [d(2-head packed), hp, t]
        QT = big.tile([P, NHP, T], bf16)
        V = big.tile([P, T // P, G, DV], bf16)  # [kv_local, kv_chunk, head, d|1]
        nc.vector.memset(V[:, :, :, D : D + 1], 1.0)  # l column (LAST: ctx
                                                # stays on partitions 0..63,
                                                # l lands on partition 64)

        # DMA order = first-use order; X slice 0 + WK first (gate the first
        # matmul), weights on the sync queue, bulk X on gpsimd in parallel.
        # ALL bulk input loads go on the sync queue, in first-use order.  The
        # gpsimd queue is reserved for small latency-sensitive transfers
        # (epilogue l/cs shifts, output tiles) so they never sit behind
        # megabytes of input stream.
        nc.sync.dma_start(tri_sb[:, 0, :], tri)
        nc.sync.dma_start(bk_sb[:], bk)
        nc.sync.dma_start(bq_sb[:], bq)
        # split the first X/W loads so kt(0,0)/q(0,0) start on their first
        # k-pair chunks instead of waiting for whole tensors
        nc.sync.dma_start(X[:, 0:2, 0:CH], x_v[:, 0:2, 0:CH])
        nc.sync.dma_start(WK[:, 0], wkT[:, 0])
        nc.sync.dma_start(X[:, 2:KC, 0:CH], x_v[:, 2:KC, 0:CH])
        for hp in range(1, NHP):
            nc.sync.dma_start(WK[:, hp], wkT[:, hp])
        nc.sync.dma_start(WQ[:, 0], wqT[:, 0])
        for hp in range(1, NHP):
            nc.sync.dma_start(WQ[:, hp], wqT[:, hp])
        nc.gpsimd.dma_start(XB[:, :, 0:CH], xb_v[:, :, 0:CH])
        nc.sync.dma_start(WV[:], wv_v)
        nc.sync.dma_start(X[:, :, ds(CH, 3 * CH)], x_v[:, :, ds(CH, 3 * CH)])
        for tb in range(1, NCH):
            nc.sync.dma_start(XB[:, :, ds(CH * tb, CH)],
                              xb_v[:, :, ds(CH * tb, CH)])
        nc.sync.dma_start(WO[:], wo_v)

        # ---------- projection tile emitters (filler units) ----------
        # Q/K projections are fp8 DoubleRow over k-chunk pairs; weights were
        # pre-scaled by 64 on the host, descaled in the epilogue.
        def kt_tile(tb, hp):
            """KT[:, hp, 512*tb : ...] <- (Wk chunk)^T @ x chunk + bk."""
            ps = psumP.tile([P, CH], f32, tag="pp", name=f"pk{tb}{hp}")
            for k in range(0, KC, 2):
                nc.tensor.matmul(ps[:], WK[:, hp, k : k + 2, :],
                                 X[:, k : k + 2, ds(CH * tb, CH)],
                                 start=(k == 0), stop=(k == KC - 2),
                                 perf_mode=DR)
            nc.vector.tensor_scalar(
                out=KT[:, hp, ds(CH * tb, CH)], in0=ps[:],
                scalar1=1.0 / 64, scalar2=bk_sb[:, hp : hp + 1],
                op0=MUL, op1=ADD)

        def q_tile(j, hp):
            ps = psumP.tile([P, CH], f32, tag="pp", name=f"pq{j}{hp}")
            for k in range(0, KC, 2):
                nc.tensor.matmul(ps[:], WQ[:, hp, k : k + 2, :],
                                 X[:, k : k + 2, ds(CH * j, CH)],
                                 start=(k == 0), stop=(k == KC - 2),
                                 perf_mode=DR)
            nc.vector.tensor_scalar(
                out=QT[:, hp, ds(CH * j, CH)], in0=ps[:],
                scalar1=1.0 / 64, scalar2=bq_sb[:, hp : hp + 1],
                op0=MUL, op1=ADD)

        def v_tile(i):
            """V rows [128i : 128(i+1)] for all 8 heads (x chunk stationary)."""
            ps = psumP.tile([P, CH], f32, tag="pp", name=f"pv{i}")
            for k in range(KC):
                nc.tensor.matmul(ps[:], XB[:, k, ts(i, P)], WV[:, k, :],
                                 start=(k == 0), stop=(k == KC - 1))
            nc.vector.tensor_copy(V[:, i, :, 0:D],
                                  ps.rearrange("p (h d) -> p h d", d=D))

        COPY = mybir.ActivationFunctionType.Copy

        def o_tile(j, m, ctxT_j):
            """Partial out rows [128m:...], q chunk j (no bias; host adds it)."""
            ps = psumP.tile([P, CH], f32, tag="pp", name=f"po{j}{m}")
            for k in range(OC):
                nc.tensor.matmul(ps[:], WO[:, k, ts(m, P)], ctxT_j[:, k, :],
                                 start=(k == 0), stop=(k == OC - 1))
            o_sb = opool.tile([P, CH], bf16, tag="o", name=f"o{j}{m}")
            if j == NCH - 1:
                # tail: the exp stream is over, ACT is idle, and DVE is
                # draining epilogues -- copy on ACT to shorten the tail
                nc.scalar.activation(o_sb[:], ps[:], COPY)
            else:
                nc.vector.tensor_copy(o_sb[:], ps[:])
            nc.sync.dma_start(out[ts(m, P), ds(CH * j, CH)], o_sb[:])

        # ---------- attention ----------
        tri_b = tri_sb[:, 0:1, :].to_broadcast((P, 2, P))

        def attn_pair(hp, j, ctxT_j, fill):
            """Head pair hp, q rows [512j : 512(j+1)], kv in [0, 512(j+1))."""
            nkv = 4 * (j + 1)
            cps = [psumC.tile([DV, CH], f32, tag="ctx", name=f"cx{j}{hp}{hh}")
                   for hh in range(2)]
            pts = {}

            def score_step(c):
                m = c - 4 * j
                qo = P * m if m >= 0 else 0
                st = psumS.tile([P, 2, CH], f32, tag="st", name=f"st{j}{hp}{c}")
                for hh in range(2):
                    nc.tensor.matmul(
                        st[:, hh, qo:],
                        KT[ds(64 * hh, 64), hp, ts(c, P)],
                        QT[ds(64 * hh, 64), hp, ds(CH * j + qo, CH - qo)],
                        start=True, stop=True)
                pt = ptpool.tile([P, 2, CH], bf16, tag="pt", name=f"pt{j}{hp}{c}")
                nc.scalar.activation(pt[:, :, qo:], st[:, :, qo:], EXP,
                                     scale=0.125)
                if m >= 0:
                    nc.vector.tensor_tensor(pt[:, :, ds(qo, P)],
                                            pt[:, :, ds(qo, P)], tri_b, MUL)
                pts[c] = (pt, qo)

            def av_step(c):
                pt, qo = pts.pop(c)
                for hh in range(2):
                    nc.tensor.matmul(
                        cps[hh][:, qo:],
                        V[:, c, 2 * hp + hh, :],
                        pt[:, hh, qo:],
                        start=(c == 0), stop=(c == nkv - 1))

            # scores pipelined one step ahead of AV; fillers paced per step
            # (emitted between the score and AV pairs so filler streaming
            # covers the AV LDWEIGHTS)
            score_step(0)
            for c in range(1, nkv):
                score_step(c)
                fill()
                av_step(c - 1)
            fill()
            av_step(nkv - 1)

            # epilogue: l sits in PSUM partition 64 of each ctx accumulator.
            # HW constraints (micro-tested): the custom DVE reciprocal and
            # gpsimd partition_broadcast only work with their source at
            # partition 0; only DMA moves data across partitions; partition
            # windows must be 32-aligned.
            # Two quick DVE copies evacuate ctx and l to SBUF so the PSUM
            # banks free up right after the last AV (the long l chain then
            # runs off the critical path): shift-DMA l to partition 0,
            # reciprocal, broadcast, normalize from the SBUF copy.
            # j=0 epilogues overlap the input DMA stream, so their small
            # transfers stay on the gpsimd queue; later ones use the (by
            # then idle) sync hardware queue for lower completion latency.
            qd = nc.gpsimd if j == 0 else nc.sync
            for hh in range(2):
                csx = cspool.tile([DV, CH], f32, tag="cx", name=f"cq{j}{hp}{hh}")
                # l row first: its shift-DMA is the longest pole of the chain
                nc.vector.tensor_copy(csx[D : D + 1, :], cps[hh][D : D + 1, :])
                l0 = lrpool.tile([1, CH], f32, tag="l0", name=f"l0{j}{hp}{hh}")
                qd.dma_start(l0[:], csx[D : D + 1, :])
                nc.vector.tensor_copy(csx[0:D, :], cps[hh][0:D, :])
                linv = lrpool.tile([1, CH], f32, tag="li", name=f"li{j}{hp}{hh}")
                nc.vector.reciprocal_approx_fast(linv[:], l0[:])
                lb = lbpool.tile([64, CH], f32, tag="lb", name=f"lb{j}{hp}{hh}")
                nc.gpsimd.partition_broadcast(lb[:], linv[:], channels=64)
                if hh == 0:
                    nc.vector.tensor_tensor(ctxT_j[0:64, hp, :],
                                            csx[0:D, :], lb[:], MUL)
                else:
                    cs1 = cspool.tile([64, CH], bf16, tag="cs",
                                      name=f"cs{j}{hp}")
                    nc.vector.tensor_tensor(cs1[:], csx[0:D, :], lb[:], MUL)
                    qd.dma_start(ctxT_j[ds(64, 64), hp, :], cs1[:])

        # ---------- schedule ----------
        # prologue: K/Q for block 0 first (their DMAs land first), then V
        for hp in range(NHP):
            kt_tile(0, hp)
        for hp in range(NHP):
            q_tile(0, hp)
        for i in range(NCH):
            v_tile(i)

        ctxT = [None] * NCH
        for j in range(NCH):
            ctxT[j] = ctxpool.tile([P, NHP, CH], bf16, tag="ctxT", name=f"cT{j}")
            # filler units due during attention chunk j
            units = []
            if j < NCH - 1:
                units += [lambda hp=hp: q_tile(j + 1, hp) for hp in range(NHP)]
                units += [lambda hp=hp: kt_tile(j + 1, hp) for hp in range(NHP)]
                units += [lambda i=i: v_tile(4 * (j + 1) + i) for i in range(4)]
            if j > 0:
                units += [lambda m=m, jj=j - 1: o_tile(jj, m, ctxT[jj])
                          for m in range(KC)]
            steps = NHP * 4 * (j + 1)
            state = {"s": 0, "f": 0}
            # hold back a few units past the last pair so PE has work during
            # the final epilogue chain of this chunk
            tail = (6 if j == NCH - 1 else 3) if units else 0
            paced = max(len(units) - tail, 0)

            def fill(units=units, steps=steps, state=state, paced=paced):
                state["s"] += 1
                want = (paced * state["s"] + steps - 1) // steps
                while state["f"] < want and state["f"] < paced:
                    units[state["f"]]()
                    state["f"] += 1

            for hp in range(NHP):
                attn_pair(hp, j, ctxT[j], fill)
            while state["f"] < len(units):
                units[state["f"]]()
                state["f"] += 1

        for m in range(KC):
            o_tile(NCH - 1, m, ctxT[NCH - 1])

    nc.compile()
    return nc


def _shard_inputs(x, Wq, bq, bk_, bv, bo, WqT, WkT, WvT, WoT):
    """Build the 8 per-core input maps. WqT is Wq.T/8; others are plain .T.

    Q/K path tensors go to the device in fp8e4m3; weights are pre-scaled by
    64 to clear the e4m3 denormal range, descaled on-device."""
    bf = ml_dtypes.bfloat16
    f8 = ml_dtypes.float8_e4m3
    tri = np.triu(np.ones((P, P), np.float32)).astype(bf)

    def perm_w(wT):  # (C, CH) -> [p, hp, k, m] (SBUF tile layout)
        return np.ascontiguousarray(
            wT.reshape(KC, P, NHP, P).transpose(1, 2, 0, 3))

    in_maps = []
    for b in range(B):
        xTb = np.ascontiguousarray(x[b].T)
        xT8 = xTb.astype(f8)
        xT16 = xTb.astype(bf)
        for g in range(2):
            sl = slice(CH * g, CH * (g + 1))
            in_maps.append({
                "xT": xT8,
                "xbT": xT16,
                "wqT": perm_w(WqT[:, sl] * 512.0).astype(f8),
                "wkT": perm_w(WkT[:, sl] * 64.0).astype(f8),
                "wvT": np.ascontiguousarray(WvT[:, sl]).astype(bf),
                "woT": np.ascontiguousarray(WoT[sl, :]).astype(bf),
                "bq": np.ascontiguousarray(bq[sl].reshape(NHP, P).T),
                "bk": np.ascontiguousarray(bk_[sl].reshape(NHP, P).T),
                "tri": tri,
            })
    return in_maps


def kernel(x, Wq, bq, Wk, bk, Wv, bv, Wo, bo):
    from concourse.bass_utils import run_bass_kernel_spmd

    x = np.asarray(x, np.float32)
    Wq = np.asarray(Wq, np.float32); bq = np.asarray(bq, np.float32)
    Wk = np.asarray(Wk, np.float32); bk = np.asarray(bk, np.float32)
    Wv = np.asarray(Wv, np.float32); bv = np.asarray(bv, np.float32)
    Wo = np.asarray(Wo, np.float32); bo = np.asarray(bo, np.float32)

    if "nc" not in _CACHE:
        _CACHE["nc"] = _build()
    nc = _CACHE["nc"]

    WqT = np.ascontiguousarray(Wq.T / 8.0)
    WkT = np.ascontiguousarray(Wk.T)
    WvT = np.ascontiguousarray(Wv.T)
    WoT = np.ascontiguousarray(Wo.T)
    in_maps = _shard_inputs(x, Wq, bq, bk, bv, bo, WqT, WkT, WvT, WoT)

    res = run_bass_kernel_spmd(nc, in_maps, core_ids=list(range(8)))
    bo_eff = (bo + Wo @ bv).astype(np.float32)
    outf = np.empty((B, T, C), np.float32)
    for b in range(B):
        o = (res.results[2 * b]["out"].astype(np.float32)
             + res.results[2 * b + 1]["out"].astype(np.float32))  # (C, T)
        outf[b] = o.T + bo_eff
    return outf
